# revision 2
# baseline (speedup 1.0000x reference)
"""SuperposedExpert, token-parallel variant: 8-way DP, no collectives.

Each core handles 256 tokens and runs ALL 4 paths over them. TT cores are
expanded to dense W1/W2 on the host (with (1 + path_weight) folded into W2).

v2 layout: gating is computed transposed (logits [token, K] via x-stationary
matmuls, softmax along the free axis), so gates become per-partition scalars
and no broadcast constants are needed. ffn2 is h-stationary with f=512
([tok, d] output): half the instruction count of the f=256 form, and each
(th, dc) drain is a single fused tensor_scalar (ps2 * gate * rden) that
overlaps the next chains, killing the output tail. Path-0 W1 rides the sync
ring alone (full HBM bw before scalar-ring traffic starts), so the PE starts
ffn1 ~8us in instead of ~36us.
"""

import numpy as np
import ml_dtypes

import concourse.bass as bass
import concourse.tile as tile
from concourse import bacc, mybir
from concourse.bass import ds, ts
from concourse.bass_utils import run_bass_kernel_spmd

BF16 = mybir.dt.bfloat16
F32 = mybir.dt.float32
AF = mybir.ActivationFunctionType
ALU = mybir.AluOpType

K = 4
D = 1024
DFF = 4096
NTOK = 2048
NCORES = 8
NTC = NTOK // NCORES   # 256 tokens per core


def _emit(nc, tc):
    xTp = nc.dram_tensor("xTp", [128, 8, NTC], BF16, kind="ExternalInput")
    # W1 packed f-quarter-major: [g][p][s][fq] so each quarter is one
    # contiguous-HBM 2MB DMA and ffn1 group g starts after (g+1) quarters
    w1p = [nc.dram_tensor(f"w1p{k}", [4, 128, 8, DFF // 4], BF16,
                          kind="ExternalInput") for k in range(K)]
    w2p = [nc.dram_tensor(f"w2p{k}", [128, 32, D], BF16, kind="ExternalInput")
           for k in range(K)]
    pbT = nc.dram_tensor("pbT", [D, K], BF16, kind="ExternalInput")
    opiece = nc.dram_tensor("opiece", [128, 2, D], BF16, kind="ExternalOutput")

    with (
        # bufs are the pacing mechanism: a prefetch DMA can only start once
        # its slot's previous tenant has been fully consumed by the PE, so
        # weight streams self-schedule just-in-time without ordering hacks.
        tc.tile_pool(name="w1pool", bufs=4) as w1pool,
        tc.tile_pool(name="w2pool", bufs=8) as w2pool,
        tc.tile_pool(name="small", bufs=1) as small,
        tc.tile_pool(name="obp", bufs=4) as obp,
        tc.tile_pool(name="pp", bufs=8, space="PSUM") as pp,
    ):
        xt_sb = small.tile([128, 8, NTC], BF16, tag="xt")
        nc.sync.dma_start(xt_sb, xTp.ap())
        pbt_sb = small.tile([128, 8, K], BF16, tag="pbt")
        nc.sync.dma_start(pbt_sb, pbT.ap().rearrange("(t p) k -> p t k", p=128))

        # w1 as independent per-quarter tiles [p, s, fq] matching the DRAM
        # packing (quarter DMAs contiguous on both sides). All of path 0 on
        # the sync ring: it is the only ring pulling at t=0, so the critical
        # first quarters get the full HBM bandwidth.
        w1q = [[w1pool.tile([128, 8, DFF // 4], BF16, tag="w1",
                            name=f"w1_{k}_{g}") for g in range(4)]
               for k in range(K)]
        for g in range(4):
            nc.sync.dma_start(w1q[0][g], w1p[0][g])

        # gating state: gexp[tok, th, k] = exp(logit), rden[tok, th] = 1/sum
        gexp = small.tile([128, 2, K], F32, tag="gexp")
        rden = small.tile([128, 2], F32, tag="rden")
        den = small.tile([128, 2], F32, tag="den")
        acc = small.tile([128, 2, D], F32, tag="acc")

        def emit_gating():
            # logits^T [tok, K] via x-stationary matmuls (f=4, trivial PE
            # time). Emitted AFTER ffn1 grp0 so the first W1 chains, not the
            # scalar engine's ~10us preamble, gate the pipeline start.
            for th in range(2):
                lg = pp.tile([128, K], F32, tag="ps", name=f"lg{th}")
                for s in range(8):
                    nc.tensor.matmul(lg, xt_sb[:, s, ts(th, 128)],
                                     pbt_sb[:, s],
                                     start=(s == 0), stop=(s == 7))
                nc.scalar.activation(gexp[:, th], lg, AF.Exp)
            for th in range(2):
                nc.vector.tensor_reduce(den[:, ds(th, 1)], gexp[:, th],
                                        mybir.AxisListType.X, ALU.add)
                nc.vector.reciprocal(rden[:, ds(th, 1)], den[:, ds(th, 1)])

        # ---------------- per-path FFN ----------------
        for k in range(K):
            ht = small.tile([128, 32, NTC], BF16, tag="ht", name=f"ht_{k}")
            w2tl = [None] * 16

            def w2_load(q):
                w2tl[q] = w2pool.tile([128, 2, D], BF16, tag="w2",
                                      name=f"w2_{k}_{q}")
                # path 0's first slices ride the sync ring FIFO-behind the
                # critical W1 load; everything later is slot-WAR paced
                eng = nc.sync if (k == 0 and q < 8) else nc.scalar
                eng.dma_start(w2tl[q], w2p[k][:, ds(2 * q, 2), :])

            for grp in range(4):
                ps1 = [pp.tile([128, NTC], F32, tag="ps",
                               name=f"f1_{k}_{grp}_{j}") for j in range(8)]
                for s in range(8):
                    for j in range(8):
                        nc.tensor.matmul(
                            ps1[j], w1q[k][grp][:, s, ts(j, 128)],
                            xt_sb[:, s],
                            start=(s == 0), stop=(s == 7),
                        )
                for j in range(8):
                    nc.scalar.activation(ht[:, grp * 8 + j], ps1[j],
                                         AF.Gelu_apprx_tanh)
                if k == 0 and grp == 0:
                    emit_gating()
                if k + 1 < K:
                    nc.scalar.dma_start(w1q[k + 1][grp], w1p[k + 1][grp])
                w2_load(2 * grp)
                w2_load(2 * grp + 1)

            # ffn2 h-stationary, f=512: out[tok, d] += h_blk^T @ w2_slice.
            # 4 chains (th x dc) step together through the w2 tiles, so w2
            # streams JIT exactly as in the f-outer form.
            ps2 = [pp.tile([128, 512], F32, tag="ps", name=f"f2_{k}_{c}")
                   for c in range(4)]
            for q in range(16):
                if q < 8:
                    w2_load(q + 8)
                for kc in range(2):
                    s2 = 2 * q + kc
                    for th in range(2):
                        for dc in range(2):
                            nc.tensor.matmul(
                                ps2[th * 2 + dc],
                                ht[:, s2, ts(th, 128)],
                                w2tl[q][:, kc, ts(dc, 512)],
                                start=(s2 == 0), stop=(s2 == 31),
                            )
            for th in range(2):
                for dc in range(2):
                    c = th * 2 + dc
                    g1 = gexp[:, th, ds(k, 1)]
                    g2 = rden[:, ds(th, 1)]
                    if k == 0:
                        nc.vector.tensor_scalar(
                            acc[:, th, ts(dc, 512)], ps2[c], g1, g2,
                            ALU.mult, ALU.mult)
                    elif k < K - 1:
                        ob = obp.tile([128, 512], F32, tag="ob",
                                      name=f"ob_{k}_{c}")
                        nc.vector.tensor_scalar(ob, ps2[c], g1, g2,
                                                ALU.mult, ALU.mult)
                        nc.vector.tensor_add(acc[:, th, ts(dc, 512)],
                                             acc[:, th, ts(dc, 512)], ob)
                    else:
                        ob = obp.tile([128, 512], F32, tag="ob",
                                      name=f"ob_{k}_{c}")
                        nc.vector.tensor_scalar(ob, ps2[c], g1, g2,
                                                ALU.mult, ALU.mult)
                        obf = obp.tile([128, 512], BF16, tag="obf",
                                       name=f"obf_{c}")
                        nc.vector.tensor_add(obf, acc[:, th, ts(dc, 512)], ob)
                        nc.sync.dma_start(opiece[:, th, ts(dc, 512)], obf)


def build(verbose=False):
    nc = bacc.Bacc("TRN2", target_bir_lowering=False, debug=False, num_devices=NCORES)
    with tile.TileContext(nc) as tc:
        _emit(nc, tc)
    nc.compile()
    return nc


def _expand_tt(core1, core2, din, dout):
    a, x, r = core1.shape
    r2, b, y = core2.shape
    m = core1.reshape(a * x, r).astype(np.float32) @ \
        core2.reshape(r2, b * y).astype(np.float32)
    w = m.reshape(a, x, b, y).transpose(0, 2, 1, 3).reshape(a * b, x * y)
    assert w.shape == (din, dout)
    return w


def make_in_maps(inputs):
    tokens = inputs["tokens"]
    bf = ml_dtypes.bfloat16
    shared = {}
    for k in range(K):
        w1 = _expand_tt(inputs["ffn1_core1"][k], inputs["ffn1_core2"][k], D, DFF)
        # [(s p), (g fq)] -> [g, p, s, fq]
        shared[f"w1p{k}"] = np.ascontiguousarray(
            w1.reshape(8, 128, 4, DFF // 4).transpose(2, 1, 0, 3)).astype(bf)
        w2 = _expand_tt(inputs["ffn2_core1"][k], inputs["ffn2_core2"][k], DFF, D)
        w2 *= (1.0 + inputs["path_weights"][k])[None, :]
        shared[f"w2p{k}"] = np.ascontiguousarray(
            w2.reshape(32, 128, D).transpose(1, 0, 2)).astype(bf)
    shared["pbT"] = np.ascontiguousarray(inputs["path_bases"].T).astype(bf)
    in_maps = []
    for c in range(NCORES):
        tok = tokens[c * NTC:(c + 1) * NTC]
        xt = np.ascontiguousarray(
            tok.T.reshape(8, 128, NTC).transpose(1, 0, 2)).astype(bf)
        m = dict(shared)
        m["xTp"] = xt
        in_maps.append(m)
    return in_maps


def assemble(results):
    out = np.empty((NTOK, D), np.float32)
    for c in range(NCORES):
        # piece [128 p, 2 th, 1024 d]; token = c*256 + th*128 + p
        piece = results[c]["opiece"].astype(np.float32)
        out[c * NTC:(c + 1) * NTC] = piece.transpose(1, 0, 2).reshape(NTC, D)
    return out


_NC = None


def run(inputs, trace=False):
    global _NC
    if _NC is None:
        _NC = build()
    res = run_bass_kernel_spmd(
        _NC, make_in_maps(inputs), core_ids=list(range(NCORES)), trace=trace
    )
    return assemble(res.results), res


def kernel(**inputs):
    out, _ = run(inputs)
    return out


# revision 7
# speedup vs baseline: 1.1231x; 1.1231x over previous
"""SuperposedExpert, token-parallel variant: 8-way DP, no collectives.

Each core handles 256 tokens and runs ALL 4 paths over them. TT cores are
expanded to dense W1/W2 on the host (with (1 + path_weight) folded into W2).

v2 layout: gating is computed transposed (logits [token, K] via x-stationary
matmuls, softmax along the free axis), so gates become per-partition scalars
and no broadcast constants are needed. ffn2 is h-stationary with f=512
([tok, d] output): half the instruction count of the f=256 form, and each
(th, dc) drain is a single fused tensor_scalar (ps2 * gate * rden) that
overlaps the next chains, killing the output tail. Path-0 W1 rides the sync
ring alone (full HBM bw before scalar-ring traffic starts), so the PE starts
ffn1 ~8us in instead of ~36us.
"""

import numpy as np
import ml_dtypes

import concourse.bass as bass
import concourse.tile as tile
from concourse import bacc, mybir
from concourse.bass import ds, ts
from concourse.bass_utils import run_bass_kernel_spmd

BF16 = mybir.dt.bfloat16
F32 = mybir.dt.float32
AF = mybir.ActivationFunctionType
ALU = mybir.AluOpType

K = 4
D = 1024
DFF = 4096
NTOK = 2048
NCORES = 8
NTC = NTOK // NCORES   # 256 tokens per core


def _emit(nc, tc):
    xTp = nc.dram_tensor("xTp", [128, 8, NTC], BF16, kind="ExternalInput")
    # W1 packed f-quarter-major: [g][p][s][fq] so each quarter is one
    # contiguous-HBM 2MB DMA and ffn1 group g starts after (g+1) quarters
    w1p = [nc.dram_tensor(f"w1p{k}", [4, 128, 8, DFF // 4], BF16,
                          kind="ExternalInput") for k in range(K)]
    w2p = [nc.dram_tensor(f"w2p{k}", [128, 32, D], BF16, kind="ExternalInput")
           for k in range(K)]
    pbT = nc.dram_tensor("pbT", [D, K], BF16, kind="ExternalInput")
    opiece = nc.dram_tensor("opiece", [128, 2, D], BF16, kind="ExternalOutput")

    with (
        # bufs are the pacing mechanism: a prefetch DMA can only start once
        # its slot's previous tenant has been fully consumed by the PE, so
        # weight streams self-schedule just-in-time without ordering hacks.
        tc.tile_pool(name="w1pool", bufs=4) as w1pool,
        # 16 bufs: path k's tile q recycles path k-1's slot q, and the
        # chain-outer last path keeps all 16 slices live at once
        tc.tile_pool(name="w2pool", bufs=16) as w2pool,
        tc.tile_pool(name="small", bufs=1) as small,
        tc.tile_pool(name="obp", bufs=4) as obp,
        tc.tile_pool(name="pp", bufs=8, space="PSUM") as pp,
    ):
        xt_sb = small.tile([128, 8, NTC], BF16, tag="xt")
        nc.sync.dma_start(xt_sb, xTp.ap())
        pbt_sb = small.tile([128, 8, K], BF16, tag="pbt")
        nc.sync.dma_start(pbt_sb, pbT.ap().rearrange("(t p) k -> p t k", p=128))

        # w1 as independent per-quarter tiles [p, s, fq] matching the DRAM
        # packing (quarter DMAs contiguous on both sides). All of path 0 on
        # the sync ring: it is the only ring pulling at t=0, so the critical
        # first quarters get the full HBM bandwidth.
        w1q = [[w1pool.tile([128, 8, DFF // 4], BF16, tag="w1",
                            name=f"w1_{k}_{g}") for g in range(4)]
               for k in range(K)]
        # groups 0-1 stream in 256KB s-slices (region-level hazards let the
        # s=0 matmuls start after the first slice, ~8us in, instead of
        # waiting for a whole 2MB quarter); groups 2-3 as single DMAs.
        for g in range(2):
            for s in range(8):
                nc.sync.dma_start(w1q[0][g][:, s], w1p[0][g][:, s, :])
        for g in range(2, 4):
            nc.sync.dma_start(w1q[0][g], w1p[0][g])

        # gating state: gexp[tok, th, k] = exp(logit), rden[tok, th] = 1/sum
        gexp = small.tile([128, 2, K], F32, tag="gexp")
        rden = small.tile([128, 2], F32, tag="rden")
        den = small.tile([128, 2], F32, tag="den")
        acc = small.tile([128, 2, D], F32, tag="acc")

        def emit_gating():
            # logits^T [tok, K] via x-stationary matmuls (f=4, trivial PE
            # time). Emitted AFTER ffn1 grp0 so the first W1 chains, not the
            # scalar engine's ~10us preamble, gate the pipeline start.
            for th in range(2):
                lg = pp.tile([128, K], F32, tag="ps", name=f"lg{th}")
                for s in range(8):
                    nc.tensor.matmul(lg, xt_sb[:, s, ts(th, 128)],
                                     pbt_sb[:, s],
                                     start=(s == 0), stop=(s == 7))
                nc.scalar.activation(gexp[:, th], lg, AF.Exp)
            for th in range(2):
                nc.vector.tensor_reduce(den[:, ds(th, 1)], gexp[:, th],
                                        mybir.AxisListType.X, ALU.add)
                nc.vector.reciprocal(rden[:, ds(th, 1)], den[:, ds(th, 1)])

        # ---------------- per-path FFN ----------------
        for k in range(K):
            ht = small.tile([128, 32, NTC], BF16, tag="ht", name=f"ht_{k}")
            w2tl = [None] * 16

            def w2_load(q):
                w2tl[q] = w2pool.tile([128, 2, D], BF16, tag="w2",
                                      name=f"w2_{k}_{q}")
                # all of path 0's w2 rides the sync ring (idle after the W1
                # ramp) so the scalar ring can dedicate itself to the w1[1]
                # prefetch; later paths are slot-WAR paced on scalar
                eng = nc.sync if k == 0 else nc.scalar
                eng.dma_start(w2tl[q], w2p[k][:, ds(2 * q, 2), :])

            for grp in range(4):
                ps1 = [pp.tile([128, NTC], F32, tag="ps",
                               name=f"f1_{k}_{grp}_{j}") for j in range(8)]
                for s in range(8):
                    for j in range(8):
                        nc.tensor.matmul(
                            ps1[j], w1q[k][grp][:, s, ts(j, 128)],
                            xt_sb[:, s],
                            start=(s == 0), stop=(s == 7),
                        )
                for j in range(8):
                    nc.scalar.activation(ht[:, grp * 8 + j], ps1[j],
                                         AF.Gelu_apprx_tanh)
                if k == 0 and grp == 0:
                    emit_gating()
                if k + 1 < K:
                    nc.scalar.dma_start(w1q[k + 1][grp], w1p[k + 1][grp])
                w2_load(2 * grp)
                w2_load(2 * grp + 1)
                if k == K - 1:
                    # no next-path W1 to prefetch: pull the whole w2 in
                    # during ffn1 so the chain-outer ffn2 can run dense
                    w2_load(8 + 2 * grp)
                    w2_load(8 + 2 * grp + 1)

            # ffn2 h-stationary, f=512: out[tok, d] += h_blk^T @ w2_slice.
            def drain(ps2c, th, dc):
                g1 = gexp[:, th, ds(k, 1)]
                g2 = rden[:, ds(th, 1)]
                c = th * 2 + dc
                if k == 0:
                    nc.vector.tensor_scalar(
                        acc[:, th, ts(dc, 512)], ps2c, g1, g2,
                        ALU.mult, ALU.mult)
                elif k < K - 1:
                    ob = obp.tile([128, 512], F32, tag="ob",
                                  name=f"ob_{k}_{c}")
                    nc.vector.tensor_scalar(ob, ps2c, g1, g2,
                                            ALU.mult, ALU.mult)
                    nc.vector.tensor_add(acc[:, th, ts(dc, 512)],
                                         acc[:, th, ts(dc, 512)], ob)
                else:
                    ob = obp.tile([128, 512], F32, tag="ob",
                                  name=f"ob_{k}_{c}")
                    nc.vector.tensor_scalar(ob, ps2c, g1, g2,
                                            ALU.mult, ALU.mult)
                    obf = obp.tile([128, 512], BF16, tag="obf",
                                   name=f"obf_{c}")
                    nc.vector.tensor_add(obf, acc[:, th, ts(dc, 512)], ob)
                    nc.sync.dma_start(opiece[:, th, ts(dc, 512)], obf)

            if k < K - 1:
                # 4 chains (th x dc) step together through the w2 tiles, so
                # w2 streams JIT; all 4 drain together at the end (they
                # overlap the next path's ffn1).
                ps2 = [pp.tile([128, 512], F32, tag="ps", name=f"f2_{k}_{c}")
                       for c in range(4)]
                for q in range(16):
                    if q < 8:
                        w2_load(q + 8)
                    for kc in range(2):
                        s2 = 2 * q + kc
                        for th in range(2):
                            for dc in range(2):
                                nc.tensor.matmul(
                                    ps2[th * 2 + dc],
                                    ht[:, s2, ts(th, 128)],
                                    w2tl[q][:, kc, ts(dc, 512)],
                                    start=(s2 == 0), stop=(s2 == 31),
                                )
                for th in range(2):
                    for dc in range(2):
                        drain(ps2[th * 2 + dc], th, dc)
            else:
                # last path: w2 fully resident, so run the 4 chains
                # sequentially and drain each one (output DMA included)
                # under the next chain's matmuls -> tiny kernel tail.
                for th in range(2):
                    for dc in range(2):
                        ps2c = pp.tile([128, 512], F32, tag="ps",
                                       name=f"f2_{k}_{th}{dc}")
                        for s2 in range(32):
                            nc.tensor.matmul(
                                ps2c, ht[:, s2, ts(th, 128)],
                                w2tl[s2 // 2][:, s2 % 2, ts(dc, 512)],
                                start=(s2 == 0), stop=(s2 == 31),
                            )
                        drain(ps2c, th, dc)


def build(verbose=False):
    nc = bacc.Bacc("TRN2", target_bir_lowering=False, debug=False, num_devices=NCORES)
    with tile.TileContext(nc) as tc:
        _emit(nc, tc)
    nc.compile()
    return nc


def _expand_tt(core1, core2, din, dout):
    a, x, r = core1.shape
    r2, b, y = core2.shape
    m = core1.reshape(a * x, r).astype(np.float32) @ \
        core2.reshape(r2, b * y).astype(np.float32)
    w = m.reshape(a, x, b, y).transpose(0, 2, 1, 3).reshape(a * b, x * y)
    assert w.shape == (din, dout)
    return w


def make_in_maps(inputs):
    tokens = inputs["tokens"]
    bf = ml_dtypes.bfloat16
    shared = {}
    for k in range(K):
        w1 = _expand_tt(inputs["ffn1_core1"][k], inputs["ffn1_core2"][k], D, DFF)
        # [(s p), (g fq)] -> [g, p, s, fq]
        shared[f"w1p{k}"] = np.ascontiguousarray(
            w1.reshape(8, 128, 4, DFF // 4).transpose(2, 1, 0, 3)).astype(bf)
        w2 = _expand_tt(inputs["ffn2_core1"][k], inputs["ffn2_core2"][k], DFF, D)
        w2 *= (1.0 + inputs["path_weights"][k])[None, :]
        shared[f"w2p{k}"] = np.ascontiguousarray(
            w2.reshape(32, 128, D).transpose(1, 0, 2)).astype(bf)
    shared["pbT"] = np.ascontiguousarray(inputs["path_bases"].T).astype(bf)
    in_maps = []
    for c in range(NCORES):
        tok = tokens[c * NTC:(c + 1) * NTC]
        xt = np.ascontiguousarray(
            tok.T.reshape(8, 128, NTC).transpose(1, 0, 2)).astype(bf)
        m = dict(shared)
        m["xTp"] = xt
        in_maps.append(m)
    return in_maps


def assemble(results):
    out = np.empty((NTOK, D), np.float32)
    for c in range(NCORES):
        # piece [128 p, 2 th, 1024 d]; token = c*256 + th*128 + p
        piece = results[c]["opiece"].astype(np.float32)
        out[c * NTC:(c + 1) * NTC] = piece.transpose(1, 0, 2).reshape(NTC, D)
    return out


_NC = None


def run(inputs, trace=False):
    global _NC
    if _NC is None:
        _NC = build()
    res = run_bass_kernel_spmd(
        _NC, make_in_maps(inputs), core_ids=list(range(NCORES)), trace=trace
    )
    return assemble(res.results), res


def kernel(**inputs):
    out, _ = run(inputs)
    return out
